# revision 4
# baseline (speedup 1.0000x reference)
"""Trainium2 Bass kernel for IR-Net style binarized conv block.

Computation (matches the reference nn.Module):
  1. Per-out-channel weight standardization -> sign -> {-1,+1}, power-of-2
     per-channel scale sw (host-side numpy; weights are tiny).
  2. ba = sign(x) (device, ScalarE Sign activation, exact in fp8).
  3. y = conv2d(ba, sign_w) * sw  -- 3x3, pad 1, stride 1. Done as 9 shifted
     matmuls over a zero-padded SBUF activation image, channels on the
     partition axis, accumulating in PSUM. Exact: products are +-1 summed in
     fp32 PSUM.
  4. Training-mode BatchNorm over the FULL batch: per-channel sum / sumsq are
     accumulated on-device, AllGather'd across the 8 cores (1KB), folded with
     sw, gamma, beta into per-channel affine a*z + b.
  5. Hardtanh clip via tensor_scalar(min,max).

Sharding: pure data parallel, batch 32 -> 4 images per core x 8 cores.

Performance notes vs the first working version:
  - WPAD=57 shared-pad layout: one zero column per padded row serves as both
    the right pad of row r and the left pad of row r+1, so each 3x3-shift
    matmul streams 8x57=456 free elements instead of 8x60=480 (PE cost is
    proportional to the moving free size).
  - The BN stats -> DRAM bounce -> AllGather chain is emitted under
    tc.high_priority() with the bounce DMAs issued from the Vector/GpSimd
    queues, so the collective fires right when the group's conv finishes
    instead of ~25us later (the trigger used to sit behind Square work on the
    Scalar queue).
  - Square (sumsq) reads the SBUF copy, not PSUM, freeing PSUM banks earlier.
  - The affine+hardtanh apply is split between the Vector and GpSimd engines
    and interleaved with the second group's conv; only the second group's
    apply (DMA-bound, ~6.4MB out) is exposed in the tail.
"""

import numpy as np
import ml_dtypes

import concourse.bacc as bacc
import concourse.bass as bass
import concourse.tile as tile
from concourse import mybir
from concourse.bass_utils import run_bass_kernel_spmd

F32 = mybir.dt.float32
BF16 = mybir.dt.bfloat16
FP8 = mybir.dt.float8e4

P = 128          # SBUF partitions
CG = 2           # channel groups: 256 channels = 2 x 128
C = 256
BN_EPS = 1e-5
N_CORES = 8
WPAD = 57        # shared-pad row length: col 0 of row r+1 == right pad of row r
HP = 64          # padded rows per image; 64*57 keeps per-image stride 16B-aligned
RT = 8           # output rows per PSUM tile (8 * 57 = 456 <= 512 fp32/bank)


def build_kernel(b_per_core=4, h=56, w=56, n_cores=N_CORES, use_fp8=True):
    """Build the per-core Bass program. Returns the compiled Bacc instance."""
    assert w + 1 <= WPAD
    assert h % RT == 0
    assert h + 2 + 2 <= HP  # 2 border rows + >=2 spare rows for fp8 slice overrun
    tiles_per_img = h // RT
    NT = b_per_core * tiles_per_img     # PSUM tiles per output-channel group
    FREEMM = RT * WPAD                  # moving free dim per matmul
    FREE = RT * w                       # useful elements per tile
    nhw_total = n_cores * b_per_core * h * w
    adt = FP8 if use_fp8 else BF16

    nc = bacc.Bacc(
        "TRN2", target_bir_lowering=False, debug=False, num_devices=n_cores
    )
    x_d = nc.dram_tensor("x", [b_per_core, C, h, w], F32, kind="ExternalInput").ap()
    w_d = nc.dram_tensor("wsgn", [P, CG, 9, C], adt, kind="ExternalInput").ap()
    coef_d = nc.dram_tensor("coef", [P, CG, 3], F32, kind="ExternalInput").ap()
    out_d = nc.dram_tensor(
        "out", [b_per_core, C, h, w], F32, kind="ExternalOutput"
    ).ap()

    mult = mybir.AluOpType.mult
    add = mybir.AluOpType.add
    subtract = mybir.AluOpType.subtract
    amin = mybir.AluOpType.min
    amax = mybir.AluOpType.max
    AF = mybir.ActivationFunctionType

    with tile.TileContext(nc) as tc:
        with (
            tc.tile_pool(name="singles", bufs=1) as singles,
            tc.tile_pool(name="xs", bufs=4) as xs_pool,
            tc.tile_pool(name="psum", bufs=8, space="PSUM") as psum_pool,
            tc.tile_pool(name="sq", bufs=2) as sq_pool,
            tc.tile_pool(name="stage", bufs=3) as stage_pool,
            tc.tile_pool(name="small", bufs=1) as small,
            tc.tile_pool(name="dram", bufs=1, space="DRAM") as dram,
        ):
            # ---- padded, binarized activations (resident) ----
            acts = singles.tile([P, CG, b_per_core, HP, WPAD], adt)
            # zero borders: top row, bottom spare rows, left column (the left
            # column of row r+1 doubles as the right pad of row r)
            nc.vector.memset(acts[:, :, :, 0, :], 0.0)
            nc.vector.memset(acts[:, :, :, h + 1 : HP, :], 0.0)
            nc.vector.memset(acts[:, :, :, :, 0:1], 0.0)

            def emit_binarize(n, slices):
                for r0, r1 in slices:
                    for a in range(CG):
                        xt = xs_pool.tile([P, 28, w], F32, tag="xstage")
                        nc.sync.dma_start(
                            out=xt[:, 0 : r1 - r0, :],
                            in_=x_d[n, a * P : (a + 1) * P, r0:r1, :],
                        )
                        nc.scalar.activation(
                            out=acts[:, a, n, 1 + r0 : 1 + r1, 1 : w + 1],
                            in_=xt[:, 0 : r1 - r0, :],
                            func=AF.Sign,
                        )

            # first 10 rows of image 0 feed conv tile 0 -- get them in flight
            # before the (bigger) weight DMA
            emit_binarize(0, [(0, 10)])

            wsb = singles.tile([P, CG, 9, C], adt)
            nc.sync.dma_start(out=wsb[:], in_=w_d)
            coef = singles.tile([P, CG, 3], F32)
            nc.sync.dma_start(out=coef[:], in_=coef_d)

            emit_binarize(0, [(10, 33), (33, h)])
            if b_per_core > 1:
                emit_binarize(1, [(0, 28), (28, h)])

            # ---- conv + BN, pipelined per output-channel group ----
            ybuf = singles.tile([P, CG, NT, FREE], F32)
            sum_p = small.tile([P, CG, NT], F32)
            sumsq_p = small.tile([P, CG, NT], F32)
            eps_t = small.tile([P, 1], F32)
            nc.vector.memset(eps_t[:], BN_EPS)

            def emit_stats_and_coef(b):
                # the whole chain is emitted at high priority so the scheduler
                # fires the collective as soon as this group's conv drains,
                # and on queues (Vector/GpSimd) that are not backlogged
                with tc.high_priority():
                    stats_b = small.tile([P, 2], F32, tag=f"stats{b}")
                    nc.vector.tensor_reduce(
                        out=stats_b[:, 0:1], in_=sum_p[:, b, :],
                        axis=mybir.AxisListType.X, op=add,
                    )
                    nc.vector.tensor_reduce(
                        out=stats_b[:, 1:2], in_=sumsq_p[:, b, :],
                        axis=mybir.AxisListType.X, op=add,
                    )
                    in_bounce = dram.tile([P, 2], F32, tag=f"inb{b}")
                    out_bounce = dram.tile([n_cores * P, 2], F32, tag=f"outb{b}")
                    nc.gpsimd.dma_start(out=in_bounce[:], in_=stats_b[:])
                    nc.gpsimd.collective_compute(
                        "AllGather",
                        mybir.AluOpType.bypass,
                        replica_groups=[list(range(n_cores))],
                        ins=[in_bounce.opt()],
                        outs=[out_bounce.opt()],
                    )
                    gst8 = small.tile([P, 2, n_cores], F32, tag=f"gst8{b}")
                    nc.gpsimd.dma_start(
                        out=gst8[:],
                        in_=out_bounce[:].rearrange("(c p) s -> p s c", c=n_cores),
                    )
                    gstats = small.tile([P, 2], F32, tag=f"gstats{b}")
                    nc.vector.tensor_reduce(
                        out=gstats[:], in_=gst8[:], axis=mybir.AxisListType.X,
                        op=add,
                    )

                    # mean = sum/nhw; ex2 = sumsq/nhw; var_y = (ex2-mean^2)*sw^2
                    # rstd = 1/sqrt(var_y+eps); a = gamma*sw*rstd; b = beta - mean*a
                    cf = small.tile([P, 6], F32, tag=f"cf{b}")
                    mean_t, ex2_t, var_t, std_t, a_t, b_t = (
                        cf[:, i : i + 1] for i in range(6)
                    )
                    nc.vector.tensor_scalar_mul(
                        mean_t, gstats[:, 0:1], 1.0 / nhw_total
                    )
                    nc.vector.tensor_scalar_mul(
                        ex2_t, gstats[:, 1:2], 1.0 / nhw_total
                    )
                    nc.vector.scalar_tensor_tensor(
                        out=var_t, in0=mean_t, scalar=mean_t, in1=ex2_t,
                        op0=mult, op1=subtract,
                    )
                    nc.vector.tensor_tensor(
                        out=var_t, in0=var_t, in1=coef[:, b, 2:3], op=mult
                    )
                    nc.vector.tensor_scalar_mul(var_t, var_t, -1.0)
                    nc.scalar.activation(
                        out=std_t, in_=var_t, func=AF.Sqrt, bias=eps_t[:],
                        scale=1.0,
                    )
                    nc.vector.reciprocal(out=std_t, in_=std_t)
                    nc.vector.tensor_tensor(
                        out=a_t, in0=coef[:, b, 0:1], in1=std_t, op=mult
                    )
                    nc.vector.scalar_tensor_tensor(
                        out=b_t, in0=mean_t, scalar=-1.0, in1=a_t,
                        op0=mult, op1=mult,
                    )
                    nc.vector.tensor_tensor(
                        out=b_t, in0=coef[:, b, 1:2], in1=b_t, op=add
                    )
                return a_t, b_t

            def emit_apply(b, n, a_t, b_t):
                # affine + hardtanh + store for one image of one channel
                # group, split between the Vector and GpSimd engines
                for t0, nt, eng, dq in (
                    (0, 4, nc.vector, nc.sync),
                    (4, tiles_per_img - 4, nc.gpsimd, nc.scalar),
                ):
                    idx = n * tiles_per_img + t0
                    stt = stage_pool.tile([P, 4 * FREE], F32, tag="aff")
                    eng.tensor_scalar(
                        out=stt[:, 0 : nt * FREE],
                        in0=ybuf[:, b, idx : idx + nt, :],
                        scalar1=a_t,
                        scalar2=b_t,
                        op0=mult,
                        op1=add,
                    )
                    st2 = stage_pool.tile([P, 4 * FREE], F32, tag="clip")
                    eng.tensor_scalar(
                        out=st2[:, 0 : nt * FREE],
                        in0=stt[:, 0 : nt * FREE],
                        scalar1=1.0,
                        scalar2=-1.0,
                        op0=amin,
                        op1=amax,
                    )
                    dq.dma_start(
                        out=out_d[
                            n,
                            b * P : (b + 1) * P,
                            t0 * RT : (t0 + nt) * RT,
                            :,
                        ],
                        in_=st2[:, 0 : nt * FREE],
                    )

            ab0 = None
            for b in range(CG):
                for n in range(b_per_core):
                    if b == 0 and n + 2 < b_per_core:
                        emit_binarize(n + 2, [(0, 28), (28, h)])
                    if b == 1:
                        # interleave group 0's apply under group 1's conv
                        emit_apply(0, n, *ab0)
                    flat = acts[:, :, n, :, :].rearrange("p g h w -> p g (h w)")
                    for t in range(tiles_per_img):
                        r0 = t * RT
                        ps = psum_pool.tile([P, FREEMM], F32, tag="ps")
                        k = 0
                        for kh in range(3):
                            for kw in range(3):
                                st = (r0 + kh) * WPAD + kw
                                nc.tensor.matmul(
                                    ps[:],
                                    lhsT=wsb[
                                        :, :, kh * 3 + kw, b * P : (b + 1) * P
                                    ],
                                    rhs=flat[:, :, st : st + FREEMM],
                                    start=(k == 0),
                                    stop=(k == 8),
                                    perf_mode=mybir.MatmulPerfMode.DoubleRow,
                                )
                                k += 1
                        idx = n * tiles_per_img + t
                        ps_v = ps[:].rearrange("p (r c) -> p r c", r=RT)[
                            :, :, 0:w
                        ]
                        # evict: copy PSUM->SBUF + per-channel sum (VectorE)
                        nc.vector.tensor_scalar(
                            out=ybuf[:, b, idx, :],
                            in0=ps_v,
                            scalar1=0.0,
                            scalar2=None,
                            op0=add,
                            op1=add,
                            accum_out=sum_p[:, b, idx : idx + 1],
                        )
                        # square + per-channel sumsq (ScalarE, from the SBUF
                        # copy so the PSUM bank frees right after the evict)
                        sqt = sq_pool.tile([P, FREE], F32, tag="sq")
                        nc.scalar.activation(
                            out=sqt[:],
                            in_=ybuf[:, b, idx, :],
                            func=AF.Square,
                            accum_out=sumsq_p[:, b, idx : idx + 1],
                        )
                ab = emit_stats_and_coef(b)
                if b == 0:
                    ab0 = ab
                else:
                    for n in range(b_per_core):
                        emit_apply(1, n, *ab)

    nc.compile()
    return nc


def prep_inputs(x, weight, gamma, beta, b_per_core, n_cores, use_fp8=True):
    """Host-side prep: weight standardization/sign/scale + sharding."""
    w64 = np.asarray(weight, dtype=np.float64)
    co = w64.shape[0]
    wf = w64.reshape(co, -1)
    mean = wf.mean(axis=1)
    bw = w64 - mean[:, None, None, None]
    std = bw.reshape(co, -1).std(axis=1, ddof=1)
    mb = np.abs(bw / std[:, None, None, None]).reshape(co, -1).mean(axis=1)
    sw = 2.0 ** np.round(np.log2(mb))
    sgn = np.sign(bw)  # {-1, 0, +1}

    # wsgn[p, a, t, co] = sgn[co, a*128+p, kh, kw]
    s = sgn.reshape(co, CG, P, 9)
    wsgn = np.ascontiguousarray(s.transpose(2, 1, 3, 0))
    adt_np = ml_dtypes.float8_e4m3 if use_fp8 else ml_dtypes.bfloat16
    wsgn = wsgn.astype(adt_np)

    ga = (np.asarray(gamma, dtype=np.float64) * sw).astype(np.float32)
    be = np.asarray(beta, dtype=np.float32)
    sw2 = (sw * sw).astype(np.float32)
    coef = np.stack(
        [
            ga.reshape(CG, P).T,       # [p, g]
            be.reshape(CG, P).T,
            sw2.reshape(CG, P).T,
        ],
        axis=-1,
    ).astype(np.float32)               # [P, CG, 3]

    x = np.asarray(x, dtype=np.float32)
    in_maps = []
    for c in range(n_cores):
        in_maps.append(
            {
                "x": np.ascontiguousarray(
                    x[c * b_per_core : (c + 1) * b_per_core]
                ),
                "wsgn": wsgn,
                "coef": coef,
            }
        )
    return in_maps


_CACHE = {}


def _get_nc(key, **kw):
    if key not in _CACHE:
        _CACHE[key] = build_kernel(**kw)
    return _CACHE[key]


def run(x, weight, gamma, beta, use_fp8=True, trace=False):
    n, c, h, w = x.shape
    b_per_core = n // N_CORES
    nc = _get_nc(
        (b_per_core, h, w, use_fp8),
        b_per_core=b_per_core,
        h=h,
        w=w,
        n_cores=N_CORES,
        use_fp8=use_fp8,
    )
    in_maps = prep_inputs(
        x, weight, gamma, beta, b_per_core, N_CORES, use_fp8=use_fp8
    )
    res = run_bass_kernel_spmd(nc, in_maps, list(range(N_CORES)), trace=trace)
    out = np.concatenate([r["out"] for r in res.results], axis=0)
    return out, res


def kernel(x, weight, gamma, beta):
    out, _ = run(x, weight, gamma, beta, use_fp8=True)
    return out
